# revision 37
# baseline (speedup 1.0000x reference)
"""Bahdanau-style additive attention kernel for Trainium2 (8 NeuronCores).

reference:
    q_h    = relu(query @ w1.T)                      (B, H)
    k_h    = relu(key @ w2.T)                        (B, T, H)
    scores = tanh(q_h[:, None, :] + k_h) @ w_out     (B, T)
    attn   = softmax(scores, axis=1)
    out    = einsum('bt,bth->bh', attn, key)         (B, H)

B=128, T=2048, H=512. Pure data parallel: 16 batch rows per core.

Device strategy (per core):
  pass 1: the dominant matmul k_h = key @ w2.T runs in fp8(e4m3) with
  DoubleRow perf mode (K=256 per matmul, 2x PE throughput, 2x less key
  DMA traffic).  The elementwise chain uses the identity
      tanh(relu(kh) + qh) = max(tanh(kh + qh), tanh(qh))
  so ScalarE applies tanh directly on PSUM with per-partition bias qh,
  and DVE applies a cheap bf16 max against tanh(qh).  scores = w_out .
  th via bf16 M=1 matmuls, drained to SBUF scores rows by small DMAs.
  softmax: two batches of 8 rows, partition-parallel.  attn transpose
  for pass 2 is done on-chip with PE transpose-mode (no DRAM trip).
  pass 2: out = attn @ key contracts over t with natural-layout bf16
  key tiles as the moving operand; interleaved with pass 1 of the next
  batch so PE / DMA / ACT / DVE all stay busy.
"""

from collections import deque

import numpy as np

import concourse.bass as bass
from concourse import bacc
import concourse.mybir as mybir
import concourse.tile as tile
from concourse import bass_utils

B, T, H = 128, 2048, 512
NCORES = 8
BPC = B // NCORES          # 16 batch rows per core
P = 128
GS = H // P                # 4 g-subtiles
TC = 512                   # pass-1 t-chunk (psum bank free dim, fp32)
NCH = T // TC              # 4 chunks
CPAIR = 2                  # chunk pairs (ACT mega-ops over [128, 1024])
TP = T // P                # 16 t-subtiles for pass 2
SMB = 4                    # softmax batch = 4 rows
NB = BPC // SMB            # 4 batches

f32 = mybir.dt.float32
bf16 = mybir.dt.bfloat16
f8e4 = mybir.dt.float8e4
AF = mybir.ActivationFunctionType
AX = mybir.AxisListType
DR = mybir.MatmulPerfMode.DoubleRow

_CACHE = {}


def _build_nc():
    nc = bacc.Bacc(trn_type="TRN2", target_bir_lowering=False)

    # [b, hs, ko, p, t] with h = hs*256 + ko*128 + p
    keyT8 = nc.dram_tensor("keyT8", [BPC, 2, 2, P, T], f8e4, kind="ExternalInput")
    key_bf = nc.dram_tensor("key_bf", [BPC, T, H], bf16, kind="ExternalInput")
    # [p, hs, ko, g] with h = hs*256 + ko*128 + p
    w2dr = nc.dram_tensor("w2dr", [P, 2, 2, H], f8e4, kind="ExternalInput")
    w1T = nc.dram_tensor("w1T", [H, H], bf16, kind="ExternalInput")
    qT = nc.dram_tensor("qT", [H, BPC], bf16, kind="ExternalInput")
    wout = nc.dram_tensor("wout", [P, GS], bf16, kind="ExternalInput")
    ident = nc.dram_tensor("ident", [SMB, SMB], bf16, kind="ExternalInput")
    assert SMB * NB == BPC
    out = nc.dram_tensor("out", [BPC, H], f32, kind="ExternalOutput")

    with tile.TileContext(nc) as tc:
        with (
            tc.tile_pool(name="const", bufs=1) as cpool,
            tc.tile_pool(name="keyT", bufs=2) as keyT_pool,
            tc.tile_pool(name="keynat", bufs=6) as keynat_pool,
            tc.tile_pool(name="th", bufs=2) as th_pool,
            tc.tile_pool(name="sm", bufs=1) as sm_pool,
            tc.tile_pool(name="ps_kh", bufs=2, space="PSUM") as ps_kh,
            tc.tile_pool(name="ps_sc", bufs=2, space="PSUM") as ps_sc,
            tc.tile_pool(name="ps_o", bufs=2, space="PSUM") as ps_o,
        ):
            # ---- constants (DMA order tuned so PE starts early: the first
            # kh matmuls need only kt(row 0) + w2; q_h/w1 can land later) ----
            w1_sb = cpool.tile([P, GS, H], bf16)       # [p, hs4, g]
            w2_sb = cpool.tile([P, 2, 2, H], f8e4)     # [p, hs, ko, g]
            qT_sb = cpool.tile([P, GS, BPC], bf16)
            wout_sb = cpool.tile([P, GS], bf16)        # [p, gs]
            id_sb = cpool.tile([SMB, SMB], bf16)

            kt0 = []
            for hs in range(2):
                k = keyT_pool.tile([P, 2, T], f8e4, tag=f"kt{hs}", name=f"kt{hs}")
                kt0.append(k)
            # row-0 keyT split in half so the first kh matmul starts ~3us
            # earlier; weights for q_h land while kh chunk-pair 0 runs
            for hs in range(2):
                nc.sync.dma_start(
                    kt0[hs][:, :, :T // 2],
                    keyT8.ap()[0, hs].rearrange("k p t -> p k t")[:, :, :T // 2])
            nc.sync.dma_start(w2_sb[:], w2dr.ap())
            nc.sync.dma_start(qT_sb[:], qT.ap().rearrange("(s p) b -> p s b", p=P))
            w1_re = w1T.ap().rearrange("(s p) g -> p s g", p=P)
            for hs in range(GS):   # split so the first q_h matmul starts early
                nc.sync.dma_start(w1_sb[:, hs, :], w1_re[:, hs, :])
            for hs in range(2):
                nc.sync.dma_start(
                    kt0[hs][:, :, T // 2:],
                    keyT8.ap()[0, hs].rearrange("k p t -> p k t")[:, :, T // 2:])
            nc.gpsimd.dma_start(wout_sb[:], wout.ap())
            nc.gpsimd.dma_start(id_sb[:], ident.ap())

            qhT = cpool.tile([P, GS, BPC], f32)        # relu(w1 q), [g, b] layout
            tqT = cpool.tile([P, GS, BPC], f32)        # tanh(qh)
            shift_sb = cpool.tile([P, 1], f32)         # softmax exp shift
            nc.vector.memset(shift_sb[:], -32.0)

            # ---- q_h (transposed layout [g, b]); hs-major so each matmul
            # only waits on its own w1 slice DMA.  One accumulation group:
            # start=True zeroes the whole 2KB PSUM bank (zero-region), so it
            # may only appear on the first matmul touching this bank ----
            ps_q = ps_kh.tile([P, 2 * TC], f32, tag="kh")
            for hs in range(GS):
                for gs in range(GS):
                    nc.tensor.matmul(
                        ps_q[:, gs * BPC:(gs + 1) * BPC],
                        lhsT=w1_sb[:, hs, gs * P:(gs + 1) * P],
                        rhs=qT_sb[:, hs, :],
                        start=(hs == 0 and gs == 0),
                        stop=(hs == GS - 1 and gs == GS - 1),
                    )
            qh_flat = qhT[:].rearrange("p s b -> p (s b)")
            nc.scalar.activation(qh_flat, ps_q[:, :GS * BPC], AF.Relu)
            nc.scalar.activation(tqT[:], qhT[:], AF.Tanh)

            # per-softmax-batch score / attn staging (ping-pong via bufs=2)
            scores_sb = [sm_pool.tile([SMB, T], f32, tag=f"scores{x % 2}",
                                      name=f"scores{x}") for x in range(NB)]
            attn_sb = [sm_pool.tile([SMB, T], bf16, tag=f"attn{x % 2}",
                                    name=f"attn{x}") for x in range(NB)]
            attnT_sb = [sm_pool.tile([P, TP, SMB], bf16, tag=f"attnT{x % 2}",
                                     name=f"attnT{x}") for x in range(NB)]

            def keyt_load(b):
                kt = []
                for hs in range(2):
                    k = keyT_pool.tile([P, 2, T], f8e4, tag=f"kt{hs}", name=f"kt{hs}")
                    nc.sync.dma_start(
                        k[:], keyT8.ap()[b, hs].rearrange("k p t -> p k t"))
                    kt.append(k)
                return kt

            # Deferred PE fill work.  kh matmuls can only run ~2 PSUM tiles
            # ahead of the slower ACT tanh drain, so between kh tiles the PE
            # is fed from these queues; items are emitted ~1 stage after
            # their inputs so their dependencies are met by the time the
            # in-order PE queue reaches them.  Scores must flush before their
            # batch's softmax; pass-2 work (and the attn transposes that gate
            # it) persists across batch boundaries so the PE has ready work
            # while the next softmax chain completes.
            fill_sc = deque()
            fill_p2 = deque()

            def drain_fill(budget, keep=0):
                while budget > 0 and (fill_sc or len(fill_p2) > keep):
                    cost, fn = (fill_sc or fill_p2).popleft()
                    fn()
                    budget -= cost

            def flush_scores():
                while fill_sc:
                    fill_sc.popleft()[1]()

            def flush_all():
                flush_scores()
                while fill_p2:
                    fill_p2.popleft()[1]()

            def kh_tile(b, cp, gs, kt):
                ps = ps_kh.tile([P, 2 * TC], f32, tag="kh", name="ps")
                # hs-outer so consecutive matmuls reuse the same stationary
                # weights (DoubleRow LDWEIGHTS is 256 columns — expensive)
                for hs in range(2):
                    for half in range(2):
                        c = cp * 2 + half
                        nc.tensor.matmul(
                            ps[:, half * TC:(half + 1) * TC],
                            lhsT=w2_sb[:, hs, :, gs * P:(gs + 1) * P],
                            rhs=kt[hs][:, :, c * TC:(c + 1) * TC],
                            start=(hs == 0),
                            stop=(hs == 1),
                            perf_mode=DR,
                        )
                th = th_pool.tile([P, 2 * TC], bf16, tag=f"th{gs}", name="th")
                # tanh(relu(kh)+qh) = max(tanh(kh+qh), tanh(qh))
                nc.scalar.activation(
                    th[:], ps[:], AF.Tanh, bias=qhT[:, gs, b:b + 1])
                nc.vector.tensor_scalar_max(th[:], th[:], tqT[:, gs, b:b + 1])
                return th

            # compute ops may only write partitions {0,32,64,96}, so scores
            # stage at partition 0 and a (partition-exempt) SBUF->SBUF DMA
            # moves the finished row into scores_sb[row].
            row_stage = {}

            def enqueue_scores(b, batch, cp, ths):
                def emit(half):
                    c = cp * 2 + half
                    if b not in row_stage:
                        row_stage[b] = th_pool.tile([1, T], f32, tag="stage",
                                                    name="stage")
                    stage = row_stage[b]
                    ps_s = ps_sc.tile([1, TC], f32, tag="sc", name="ps_s")
                    for gs in range(GS):
                        nc.tensor.matmul(
                            ps_s[:],
                            lhsT=wout_sb[:, gs:gs + 1],
                            rhs=ths[gs][:, half * TC:(half + 1) * TC],
                            start=(gs == 0),
                            stop=(gs == GS - 1),
                        )
                    nc.vector.tensor_copy(
                        stage[0:1, c * TC:(c + 1) * TC], ps_s[:])
                    if c == NCH - 1:
                        nc.gpsimd.dma_start(
                            scores_sb[batch][b % SMB:b % SMB + 1, :],
                            row_stage.pop(b)[:])
                for half in range(2):
                    fill_sc.append((853, lambda h=half: emit(h)))

            def softmax(batch):
                # scores are tanh-bounded: |s| <= sum|w_out| ~= 18, so a fixed
                # shift of -32 keeps exp() in normal fp32 range with no
                # max-reduction (softmax is shift-invariant).
                sc = scores_sb[batch]
                expv = sm_pool.tile([SMB, T], f32, tag="expv", name="expv")
                sums = sm_pool.tile([SMB, 1], f32, tag="sums", name="sums")
                nc.scalar.activation(
                    expv[:], sc[:], AF.Exp, bias=shift_sb[:SMB, :],
                    accum_out=sums[:])
                inv = sm_pool.tile([SMB, 1], f32, tag="inv", name="inv")
                nc.vector.reciprocal(inv[:], sums[:])
                nc.vector.tensor_scalar_mul(attn_sb[batch][:], expv[:], inv[:])

            def transpose_attn(batch):
                # transpose attn on-chip: [SMB, T] -> [P, TP, SMB].  Emitted
                # one pass-1 row into the next batch so these PE ops never
                # block ready matmuls behind the softmax dependency wall.
                ps_tr = ps_sc.tile([P, TP, SMB], bf16, tag="sc", name="ps_tr")
                for tp in range(TP):
                    # one accumulation group (disjoint slices of one bank):
                    # start=True zeroes the whole bank, so first matmul only
                    nc.tensor.matmul(
                        ps_tr[:, tp, :],
                        lhsT=attn_sb[batch][:, tp * P:(tp + 1) * P],
                        rhs=id_sb[:],
                        is_transpose=True,
                        start=(tp == 0),
                        stop=(tp == TP - 1),
                    )
                nc.vector.tensor_copy(attnT_sb[batch][:], ps_tr[:])

            def kn_load(b):
                kn = keynat_pool.tile([P, TP, H], bf16, tag="kn", name="kn")
                nc.sync.dma_start(
                    kn[:], key_bf.ap()[b].rearrange("(c p) h -> p c h", p=P))
                return kn

            def enqueue_p2(b, batch, kn):
                ps_out = ps_o.tile([1, H], f32, tag="o", name="ps_out")
                CH = 4
                def emit(lo):
                    for tp in range(lo, lo + CH):
                        nc.tensor.matmul(
                            ps_out[:],
                            lhsT=attnT_sb[batch][:, tp, b % SMB:b % SMB + 1],
                            rhs=kn[:, tp, :],
                            start=(tp == 0),
                            stop=(tp == TP - 1),
                        )
                    if lo + CH == TP:
                        ostage = th_pool.tile([1, H], f32, tag="ostage",
                                              name="ostage")
                        nc.vector.tensor_copy(ostage[:], ps_out[:])
                        nc.gpsimd.dma_start(out.ap()[b:b + 1, :], ostage[:])
                for lo in range(0, TP, CH):
                    fill_p2.append((CH * 213, lambda l=lo: emit(l)))

            # 4 batches of 4 rows: pass2 of batch i interleaves with pass1 of
            # batch i+1 so PE stays fed; kn loads decoupled (prefetch 1 batch).
            kn_tiles = {}
            for batch in range(NB):
                for j in range(SMB):
                    b = batch * SMB + j
                    kt = kt0 if b == 0 else keyt_load(b)
                    # reserve some pass-2 backlog near the end so the PE has
                    # ready work while the final softmax chain completes
                    keep = 7 if (batch == NB - 1 and j >= 2) else 0
                    for cp in range(CPAIR):
                        ths = []
                        for gs in range(GS):
                            ths.append(kh_tile(b, cp, gs, kt))
                            drain_fill(620, keep)
                        enqueue_scores(b, batch, cp, ths)
                        if j == 0 and batch >= 1 and cp == 0:
                            # emitted mid-row: exp sits behind only ~4 tanh
                            # ops in the ACT queue and its input DMA is done,
                            # so attn is ready before the transposes drain
                            flush_scores()
                            softmax(batch - 1)
                        # transposes enqueued a row after softmax emission so
                        # the softmax chain has completed by the time the
                        # in-order PE queue reaches them
                        if batch >= 1 and j == 1 and cp == 0:
                            fill_p2.append(
                                (700, lambda bt=batch - 1: transpose_attn(bt)))
                            for pj in range(SMB):
                                pb = (batch - 1) * SMB + pj
                                enqueue_p2(pb, batch - 1, kn_tiles.pop(pb))
                    kn_tiles[b] = kn_load(b)
            flush_scores()
            softmax(NB - 1)
            fill_p2.append((700, lambda: transpose_attn(NB - 1)))
            for j in range(SMB):
                b = (NB - 1) * SMB + j
                enqueue_p2(b, NB - 1, kn_tiles.pop(b))
            flush_all()

    nc.compile()
    return nc


def kernel(query, key, w1, w2, w_out):
    query = np.asarray(query, dtype=np.float32)
    key = np.asarray(key, dtype=np.float32)
    w1 = np.asarray(w1, dtype=np.float32)
    w2 = np.asarray(w2, dtype=np.float32)
    w_out = np.asarray(w_out, dtype=np.float32)

    if "nc" not in _CACHE:
        _CACHE["nc"] = _build_nc()
    nc = _CACHE["nc"]

    np_f8 = mybir.dt.np(f8e4)
    np_bf16 = mybir.dt.np(bf16)

    # w2dr[p, hs, ko, g] = w2[g, hs*256 + ko*128 + p]
    w2T = np.ascontiguousarray(w2.T)                       # [h, g]
    w2dr = np.ascontiguousarray(
        w2T.reshape(2, 2, P, H).transpose(2, 0, 1, 3))     # [p, hs, ko, g]
    w2dr = np.clip(w2dr, -240, 240).astype(np_f8)
    w1T = np.ascontiguousarray(w1.T).astype(np_bf16)
    wout_pre = np.ascontiguousarray(w_out.reshape(GS, P).T).astype(np_bf16)
    qT = np.ascontiguousarray(query.T).astype(np_bf16)     # [H, B]
    ident = np.eye(SMB, dtype=np.float32).astype(np_bf16)

    in_maps = []
    for c in range(NCORES):
        sl = slice(c * BPC, (c + 1) * BPC)
        key_c = key[sl]
        keyT8 = np.ascontiguousarray(
            key_c.transpose(0, 2, 1).reshape(BPC, 2, 2, P, T))
        keyT8 = np.clip(keyT8, -240, 240).astype(np_f8)
        in_maps.append({
            "keyT8": keyT8,
            "key_bf": np.ascontiguousarray(key_c).astype(np_bf16),
            "w2dr": w2dr,
            "w1T": w1T,
            "qT": np.ascontiguousarray(qT[:, sl]),
            "wout": wout_pre,
            "ident": ident,
        })

    _CACHE["in_maps"] = in_maps
    import os
    trace = bool(int(os.environ.get("BENCH_TRACE", "0")))
    res = None
    last_exc = None
    for _attempt in range(3):
        try:
            res = bass_utils.run_bass_kernel_spmd(
                nc, in_maps, core_ids=list(range(NCORES)), trace=trace)
            break
        except Exception as e:  # transient device wedge: retry
            last_exc = e
            import time as _time
            _time.sleep(2.0)
    if res is None:
        raise last_exc
    if trace:
        print(f"HW exec time: {res.exec_time_ns} ns")
        if res.instructions_and_trace:
            print("trace:", res.instructions_and_trace[1])
        _CACHE["res"] = res
    out = np.concatenate([r["out"] for r in res.results], axis=0)
    return out.astype(np.float32)
